# revision 46
# baseline (speedup 1.0000x reference)
"""Multi-head attention (b=2, n=2048, dim=1024, h=16) on 8 TRN2 NeuronCores.

Sharding: tensor-parallel over heads x data-parallel over batch.
Core c handles batch c//4 and head-group c%4 (4 heads of 64 dims each).
Each core computes its QKV projection slice, local attention for its 4
heads, and a partial output projection (row-slice of W_out); the host
sums the 4 partials per batch and adds b_out.

Per-core kernel layout (all matmul operands bf16, fp32 PSUM accumulate):
  - x^T is staged in SBUF as [128, kt*2048+i] so it serves both as the
    moving operand of the q/k projections (q^T/k^T in [d, n] layout) and
    as the stationary operand of the v projection (V in [n, d] layout).
  - scores are computed transposed (S^T[j, i] = k_j . q_i) so softmax's
    denominator comes free from the PV matmul: V is augmented with a
    ones column, so O_aug = [V|1]^T @ P gives O^T rows 0..63 and the
    softmax denominator in row 64.  exp() runs on ScalarE out of PSUM.
  - normalization multiplies O^T by 1/denom broadcast across partitions
    via a K=1 ones-matmul, then the W_out row-slice matmul produces the
    partial output in [i, dim] layout for direct DMA out.
"""

import sys

sys.path.insert(0, "/opt/trn_rl_repo")

import numpy as np
import ml_dtypes

B, N, DIM, H = 2, 2048, 1024, 16
D = DIM // H            # 64 head dim
NCORES = 8
HPC = 4                 # heads per core
DL = HPC * D            # 256 local head dims per core
KT = DIM // 128         # 8 contraction tiles for projections
NT = N // 128           # 16 n tiles
QW = KT * 512           # columns per i-quarter in the packed x^T layout
SCALE = D ** -0.5       # 0.125, folded into Wq host-side (exact power of 2)

_cached_nc = None
_ldw_patched = False


def _enable_ldw_opt():
    """walrus ships with --enable-ldw-opt=false; enabling it lets LDWEIGHTS
    double-buffer into the background weight slots so back-to-back matmuls
    pipeline. Correctness is verified by the caller's rel-err check."""
    global _ldw_patched
    if _ldw_patched:
        return
    from concourse import bass_utils as bu

    orig = bu.run_command

    # NOTE: tried flipping --enable-ldw-opt=true: walrus codegen rejects it
    # (visitInstLdweights error), so LDWEIGHTS stays per-matmul.
    _ = orig
    _ldw_patched = True


def _build_nc():
    _enable_ldw_opt()
    from concourse import bacc, mybir, tile

    bf16 = mybir.dt.bfloat16
    f32 = mybir.dt.float32
    Exp = mybir.ActivationFunctionType.Exp
    Recip = mybir.ActivationFunctionType.Reciprocal
    mult = mybir.AluOpType.mult

    nc = bacc.Bacc(None, target_bir_lowering=False, debug=False)

    xt = nc.dram_tensor("xt", [128, KT * N], bf16, kind="ExternalInput")
    wq = nc.dram_tensor("wq", [128, KT * DL], bf16, kind="ExternalInput")
    wk = nc.dram_tensor("wk", [128, KT * DL], bf16, kind="ExternalInput")
    wv = nc.dram_tensor("wv", [128, KT * DL], bf16, kind="ExternalInput")
    wo = nc.dram_tensor("wo", [128, 2 * DIM], bf16, kind="ExternalInput")
    out = nc.dram_tensor("out", [N, DIM], bf16, kind="ExternalOutput")

    with tile.TileContext(nc) as tc:
        with (
            tc.tile_pool(name="wpool", bufs=1) as wpool,
            tc.tile_pool(name="qkvpool", bufs=1) as qkvpool,
            tc.tile_pool(name="ppool", bufs=24) as ppool,
            tc.tile_pool(name="opool", bufs=1) as opool,
            tc.tile_pool(name="outpool", bufs=6) as outpool,
        ):
            # ---- input DMAs (weights first so projections start ASAP) ----
            wq_sb = wpool.tile([128, KT * DL], bf16, tag="wq_sb")
            wk_sb = wpool.tile([128, KT * DL], bf16, tag="wk_sb")
            wv_sb = wpool.tile([128, KT * DL], bf16, tag="wv_sb")
            wo_sb = wpool.tile([128, 2 * DIM], bf16, tag="wo_sb")
            nc.sync.dma_start(out=wq_sb[:], in_=wq[:])
            nc.sync.dma_start(out=wk_sb[:], in_=wk[:])
            # x^T arrives i-quarter-major: the prelude projections only read
            # quarters 0/1, so the first score tile unlocks after 2MB not 4MB
            xt_sb = wpool.tile([128, KT * N], bf16, tag="xt_sb")
            for q in (0, 1):
                nc.sync.dma_start(
                    out=xt_sb[:, q * QW:(q + 1) * QW], in_=xt[:, q * QW:(q + 1) * QW]
                )
            nc.sync.dma_start(out=wv_sb[:], in_=wv[:])
            for q in (2, 3):
                nc.sync.dma_start(
                    out=xt_sb[:, q * QW:(q + 1) * QW], in_=xt[:, q * QW:(q + 1) * QW]
                )
            nc.sync.dma_start(out=wo_sb[:], in_=wo[:])
            ones_sb = wpool.tile([1, D], bf16, tag="ones_sb")
            nc.vector.memset(ones_sb[:], 1.0)
            warm_sb = wpool.tile([128, 512], bf16, tag="warm_sb")
            nc.vector.memset(warm_sb[:], 0.0)

            # q^T/k^T in [d_local, n] layout: two tiles of [128, N] (2 heads each)
            qT = [qkvpool.tile([128, N], bf16, tag=f"qT{i}", name=f"qT{i}") for i in range(2)]
            kT = [qkvpool.tile([128, N], bf16, tag=f"kT{i}", name=f"kT{i}") for i in range(2)]
            # V augmented with ones column: per n-tile jt, per head h the
            # columns [jt*260 + h*65, jt*260 + h*65 + 65) hold [V_h | 1].
            vaug = qkvpool.tile([128, NT * (DL + HPC)], bf16, tag="vaug")
            nc.vector.memset(vaug[:], 1.0)

            # unnormalized O^T + denom, per head; normalized O^T head-pairs
            osb = [opool.tile([D, N], bf16, tag=f"osb{h}", name=f"osb{h}") for h in range(HPC)]
            OT = [opool.tile([128, N], bf16, tag=f"OT{i}", name=f"OT{i}") for i in range(2)]
            recips = [opool.tile([1, N], bf16, tag=f"recip{h}", name=f"recip{h}") for h in range(HPC)]

            # One PSUM scope for projections + attention so they overlap.
            # The "st" tag (score tiles, projection groups, bcast, wout all
            # share its 3 rotating [128,1024] slots) uses 6 banks; the two
            # [65,512] PV accumulators use the remaining 2 of 8 banks.
            with (
                tc.tile_pool(name="stps", bufs=3, space="PSUM") as stps,
                tc.tile_pool(name="oaps", bufs=2, space="PSUM") as oaps,
            ):
                def proj_v():
                    for jt in range(NT):
                        pj = stps.tile([128, 512], f32, tag="st", name="pj")
                        for kt in range(KT):
                            nc.tensor.matmul(
                                pj[:, 0:DL],
                                xt_sb[:, (jt // 4) * QW + kt * 512 + (jt % 4) * 128:
                                       (jt // 4) * QW + kt * 512 + (jt % 4) * 128 + 128],
                                wv_sb[:, kt * DL:(kt + 1) * DL],
                                start=(kt == 0),
                                stop=(kt == KT - 1),
                            )
                        base = jt * (DL + HPC)
                        for h in range(HPC):
                            nc.vector.tensor_copy(
                                out=vaug[:, base + h * 65: base + h * 65 + D],
                                in_=pj[:, h * D:(h + 1) * D],
                            )

                def normalize(h, chunks=(0, 1, 2, 3)):
                    ht, ho = h // 2, (h % 2) * D
                    for c in chunks:
                        bc = stps.tile([D, 512], f32, tag="st", name="bc")
                        nc.tensor.matmul(
                            bc[:],
                            ones_sb[:],
                            recips[h][:, c * 512:(c + 1) * 512],
                            start=True,
                            stop=True,
                        )
                        nc.vector.tensor_tensor(
                            out=OT[ht][ho:ho + D, c * 512:(c + 1) * 512],
                            in0=osb[h][0:D, c * 512:(c + 1) * 512],
                            in1=bc[:],
                            op=mult,
                        )

                # attention stages s=(h, half); PV for stage s-1 is issued
                # interleaved with stage s's ST/exp so PE never waits on the
                # current stage's exp.
                stages = [(h, half) for h in range(HPC) for half in range(2)]
                p_tiles = {}
                oa_tiles = {}

                def issue_st_exp(s, jt):
                    h, half = stages[s]
                    ht, ho = h // 2, (h % 2) * D
                    st = stps.tile([128, 1024], f32, tag="st", name="st")
                    for c2 in range(2):
                        i0 = half * 1024 + c2 * 512
                        nc.tensor.matmul(
                            st[:, c2 * 512:(c2 + 1) * 512],
                            kT[ht][ho:ho + D, jt * 128:(jt + 1) * 128],
                            qT[ht][ho:ho + D, i0:i0 + 512],
                            start=True,
                            stop=True,
                        )
                    p_t = ppool.tile([128, 1024], bf16, tag="p", name="p")
                    nc.scalar.activation(out=p_t[:], in_=st[:], func=Exp)
                    p_tiles[(s, jt)] = p_t

                def issue_pv(s, jt):
                    h, half = stages[s]
                    if jt == 0:
                        oa_tiles[s] = [
                            oaps.tile([65, 512], f32, tag="oa", name="oa")
                            for _ in range(2)
                        ]
                    p_t = p_tiles.pop((s, jt))
                    vbase = jt * (DL + HPC) + h * 65
                    for c2 in range(2):
                        nc.tensor.matmul(
                            oa_tiles[s][c2][:],
                            vaug[:, vbase: vbase + 65],
                            p_t[:, c2 * 512:(c2 + 1) * 512],
                            start=(jt == 0),
                            stop=(jt == NT - 1),
                        )

                def finish_stage(s):
                    # copy stage-s accumulators out (O^T rows + denom row) and
                    # take 1/denom straight from PSUM; normalize when a head's
                    # second half lands
                    h, half = stages[s]
                    for c2, oa in enumerate(oa_tiles.pop(s)):
                        i0 = half * 1024 + c2 * 512
                        nc.vector.tensor_copy(out=osb[h][:, i0:i0 + 512], in_=oa[0:D, :])
                        # custom-DVE ops cannot shift partitions: compute
                        # 1/denom in place at partition 64, then cast-copy
                        # (plain copy can shift) to the bf16 row at partition 0
                        rec = opool.tile([65, 512], f32, tag="rectmp", name="rec", bufs=3)
                        nc.vector.reciprocal_approx_fast(out=rec[:], in_=oa[:])
                        nc.vector.tensor_copy(
                            out=recips[h][:, i0:i0 + 512], in_=rec[D:D + 1, :]
                        )
                    if half == 1:
                        if h == HPC - 1:
                            normalize(h, (2, 3))
                        else:
                            # spread this head's normalize chunks through the
                            # next stage's jt loop: the bcast matmuls then
                            # borrow score slots one at a time instead of
                            # congesting the stage boundary
                            for c in range(4):
                                fill[(s + 2, 4 * c + 2)] = (
                                    lambda h=h, c=c: normalize(h, (c,))
                                )
                    elif h == HPC - 1:
                        # head 3's first-half inputs are final a stage early;
                        # normalizing here shortens the tail's critical path
                        normalize(h, (0, 1))

                def proj_v_group(jt):
                    pj = stps.tile([128, 512], f32, tag="st", name="pj")
                    for kt in range(KT):
                        nc.tensor.matmul(
                            pj[:, 0:DL],
                            xt_sb[:, (jt // 4) * QW + kt * 512 + (jt % 4) * 128:
                                       (jt // 4) * QW + kt * 512 + (jt % 4) * 128 + 128],
                            wv_sb[:, kt * DL:(kt + 1) * DL],
                            start=(kt == 0),
                            stop=(kt == KT - 1),
                        )
                    base = jt * (DL + HPC)
                    for h in range(HPC):
                        nc.vector.tensor_copy(
                            out=vaug[:, base + h * 65: base + h * 65 + D],
                            in_=pj[:, h * D:(h + 1) * D],
                        )

                def proj_qk_group(w_sb, dest, mt, c, part=None):
                    # part=0 emits the first half of the contraction, part=1
                    # the second half + copy; None emits everything
                    if part != 1:
                        self_pj = stps.tile([128, 512], f32, tag="st", name="pj")
                        proj_qk_group.pj = self_pj
                    pj = proj_qk_group.pj
                    kts = {0: range(0, KT // 2), 1: range(KT // 2, KT), None: range(KT)}[part]
                    for kt in kts:
                        nc.tensor.matmul(
                            pj[:],
                            w_sb[:, kt * DL + mt * 128: kt * DL + mt * 128 + 128],
                            xt_sb[:, c * QW + kt * 512: c * QW + (kt + 1) * 512],
                            start=(kt == 0),
                            stop=(kt == KT - 1),
                        )
                    if part != 0:
                        nc.vector.tensor_copy(
                            out=dest[mt][:, c * 512:(c + 1) * 512], in_=pj[:]
                        )

                # warm-up: dependency-free matmuls on memset scratch run
                # during the input-DMA wait so the PE clock-gate (HAM) is
                # already at full rate when the real projections start
                wm = stps.tile([128, 512], f32, tag="st", name="wm")
                for i in range(22):
                    nc.tensor.matmul(
                        wm[:],
                        warm_sb[:, 0:128],
                        warm_sb[:],
                        start=(i == 0),
                        stop=(i == 21),
                    )

                # emission: minimal prelude (3 groups unlock stage-0's first
                # STs), every other projection group spread as just-in-time
                # PE filler across stages 0-2 so the exp stream starts early
                # and stays fed.
                proj_qk_group(wk_sb, kT, 0, 0, None)
                proj_qk_group(wq_sb, qT, 0, 0, None)
                proj_qk_group(wq_sb, qT, 0, 1, None)
                fill = {
                    (0, 1): lambda: proj_qk_group(wk_sb, kT, 0, 1, None),
                    (0, 4): lambda: proj_qk_group(wk_sb, kT, 0, 2, None),
                    (0, 7): lambda: proj_qk_group(wk_sb, kT, 0, 3, None),
                    (0, 10): lambda: proj_qk_group(wq_sb, qT, 0, 2, None),
                    (0, 12): lambda: proj_qk_group(wq_sb, qT, 0, 3, None),
                }
                for j in range(3):
                    fill[(0, 13 + j)] = (lambda j=j: proj_v_group(j))
                for j in range(13):
                    fill[(1, j)] = (lambda j=j: proj_v_group(j + 3))
                mt1 = [(wq_sb, qT, 1, c) for c in range(4)] + [(wk_sb, kT, 1, c) for c in range(4)]
                for i in range(8):
                    fill[(2, 2 * i)] = (lambda i=i: proj_qk_group(*mt1[i], None))
                s_last = len(stages) - 1
                for s in range(len(stages)):
                    for jt in range(NT):
                        issue_st_exp(s, jt)
                        if (s, jt) in fill:
                            fill[(s, jt)]()
                        # PV stream runs 2 j-tiles behind the ST stream: the
                        # first PV of a stage (whose accumulator allocation
                        # WAR-waits on the previous stage's DVE copies) then
                        # has ~2 exp-periods of slack, so the PE never idles
                        # long enough at a boundary to re-throttle the clock
                        if s > 0 and jt >= 3:
                            issue_pv(s - 1, jt - 3)
                    if s > 0:
                        for tail_jt in (NT - 3, NT - 2, NT - 1):
                            issue_pv(s - 1, tail_jt)
                        finish_stage(s - 1)
                for jt in range(NT):
                    issue_pv(s_last, jt)
                # split last finish: copies/recips, then the first 8 output
                # rows (their OT columns are already final), THEN head-3's
                # remaining normalize chunks, then the rest — the PE stream
                # never waits on the DVE recip chain this way
                h_l, half_l = stages[s_last]
                for c2, oa in enumerate(oa_tiles.pop(s_last)):
                    i0 = half_l * 1024 + c2 * 512
                    nc.vector.tensor_copy(out=osb[h_l][:, i0:i0 + 512], in_=oa[0:D, :])
                    rec = opool.tile([65, 512], f32, tag="rectmp", name="rec", bufs=3)
                    nc.vector.reciprocal_approx_fast(out=rec[:], in_=oa[:])
                    nc.vector.tensor_copy(
                        out=recips[h_l][:, i0:i0 + 512], in_=rec[D:D + 1, :]
                    )

                def wout_rows(its):
                    # output projection, bf16 partials (host sums in fp32)
                    for it in its:
                        o_sb = outpool.tile([128, 1024], bf16, tag="o_sb", name="o_sb")
                        for cc in range(2):
                            wp = stps.tile([128, 512], f32, tag="st", name="wp")
                            for kt in range(2):
                                nc.tensor.matmul(
                                    wp[:],
                                    OT[kt][:, it * 128:(it + 1) * 128],
                                    wo_sb[:, kt * DIM + cc * 512: kt * DIM + (cc + 1) * 512],
                                    start=(kt == 0),
                                    stop=(kt == 1),
                                )
                            if cc == 0:
                                nc.vector.tensor_copy(
                                    out=o_sb[:, cc * 512:(cc + 1) * 512], in_=wp[:]
                                )
                            else:
                                nc.scalar.copy(
                                    out=o_sb[:, cc * 512:(cc + 1) * 512], in_=wp[:]
                                )
                        nc.sync.dma_start(
                            out=out[it * 128:(it + 1) * 128, :], in_=o_sb[:]
                        )

                wout_rows(range(0, 8))
                normalize(h_l, (2, 3))
                wout_rows(range(8, 16))

    nc.compile()
    return nc


def _get_nc():
    global _cached_nc
    if _cached_nc is None:
        _cached_nc = _build_nc()
    return _cached_nc


def _pack_kt(a):
    """[K, M] -> [128, (K//128)*M] with [p, kt*M + m] = a[kt*128 + p, m]."""
    k, m = a.shape
    return np.ascontiguousarray(
        a.reshape(k // 128, 128, m).transpose(1, 0, 2).reshape(128, -1)
    )


def _make_in_maps(x, W_qkv, W_out):
    bf = ml_dtypes.bfloat16
    in_maps = []
    for c in range(NCORES):
        b, g = c // HPC, c % HPC
        xT = np.ascontiguousarray(x[b].T)  # [DIM, N] fp32
        # [p, q*QW + kt*512 + ii] = xT[kt*128+p, q*512+ii]  (i-quarter-major)
        xtq = xT.reshape(KT, 128, 4, 512).transpose(1, 2, 0, 3).reshape(128, 4 * QW)
        in_maps.append({
            "xt": np.ascontiguousarray(xtq).astype(bf),
            "wq": _pack_kt(W_qkv[:, g * DL:(g + 1) * DL] * SCALE).astype(bf),
            "wk": _pack_kt(W_qkv[:, DIM + g * DL: DIM + (g + 1) * DL]).astype(bf),
            "wv": _pack_kt(W_qkv[:, 2 * DIM + g * DL: 2 * DIM + (g + 1) * DL]).astype(bf),
            "wo": _pack_kt(W_out[g * DL:(g + 1) * DL, :]).astype(bf),
        })
    return in_maps


def _run(x, W_qkv, W_out, b_out, trace=False):
    from concourse.bass_utils import run_bass_kernel_spmd

    nc = _get_nc()
    in_maps = _make_in_maps(x, W_qkv, W_out)
    res = run_bass_kernel_spmd(nc, in_maps, core_ids=list(range(NCORES)), trace=trace)
    y = np.zeros((B, N, DIM), np.float32)
    for c in range(NCORES):
        y[c // HPC] += res.results[c]["out"].astype(np.float32)
    y += b_out.astype(np.float32)[None, None, :]
    return y, res


def _numpy_reference(x, mask, W_qkv, W_out, b_out):
    """Slow exact fallback (only used if mask is not all-True)."""
    b, n, dim = x.shape
    d = dim // H
    qkv = x @ W_qkv
    q, k, v = np.split(qkv, 3, axis=-1)
    th = lambda t: t.reshape(b, n, H, d).transpose(0, 2, 1, 3)
    q, k, v = th(q), th(k), th(v)
    dots = np.einsum('bhid,bhjd->bhij', q, k) * (d ** -0.5)
    dots = np.where(mask[:, None, None, :], dots, -np.finfo(np.float32).max)
    dots -= dots.max(-1, keepdims=True)
    e = np.exp(dots)
    attn = e / e.sum(-1, keepdims=True)
    o = np.einsum('bhij,bhjd->bhid', attn, v)
    o = o.transpose(0, 2, 1, 3).reshape(b, n, dim)
    return o @ W_out + b_out


def kernel(x, mask, W_qkv, W_out, b_out):
    x = np.asarray(x, np.float32)
    mask = np.asarray(mask)
    W_qkv = np.asarray(W_qkv, np.float32)
    W_out = np.asarray(W_out, np.float32)
    b_out = np.asarray(b_out, np.float32)
    assert x.shape == (B, N, DIM) and W_qkv.shape == (DIM, 3 * DIM)
    if not mask.all():
        return _numpy_reference(x, mask, W_qkv, W_out, b_out).astype(np.float32)
    y, _ = _run(x, W_qkv, W_out, b_out, trace=False)
    return y
